# revision 24
# baseline (speedup 1.0000x reference)
"""GraphNorm Trainium2 kernel (v4: channel-major fp16, fold-tree sums,
software-pipelined DMA issue).

out = weight * (x - mean[batch]*ms) / sqrt(var[batch]+eps) + bias,
per-graph mean/var over nodes; var = E[x^2] - (2*ms - ms^2) * mean^2.

Strategy (8 cores, SPMD, one shared program):
  - Host casts x to fp16, lays it out CHANNEL-MAJOR per core
    [C=128 partitions, padded nodes]; each graph ("slot") is a
    contiguous span padded with zeros to a multiple of 128. Slot
    lengths are uniform across cores (max over cores after a snake
    deal of size-sorted graphs) so one program serves all 8 cores.
  - Slots are sorted by length, so each chunk consists of a few
    groups of EQUAL-length slots. Per group, sum(x) is a fold tree:
    strided [128, ns, L] tensor_tensor adds halve L (2x fp16 DVE),
    then one tensor_reduce finishes (avoids the 1x-rate per-slot
    accumulate path).
  - sum(x^2) via ACT Square+accum_out per slot, concurrent with the
    DVE folds (squares scratch into OUT, later overwritten by apply).
  - Apply is one fused DVE tensor_scalar (x*W + B) per slot (4x fp16).
  - Chunk loads are emitted PREFETCH chunks ahead of the compute so
    the in-order sync sequencer never parks a load behind a store's
    semaphore wait.
"""

import sys

sys.path.insert(0, "/opt/trn_rl_repo")

import numpy as np

import concourse.bass as bass
import concourse.bacc as bacc
import concourse.tile as tile
from concourse import mybir
from concourse.bass_utils import run_bass_kernel_spmd

f32 = mybir.dt.float32
f16 = mybir.dt.float16

N, C, B = 500000, 128, 512
EPS = 1e-5
NCORES = 8
CHUNK_MAX = 16384
FIRST_CAP = 4096   # small first chunk -> fast pipeline fill
LAST_CAP = 3072    # small last chunk -> fast pipeline drain
FOLD_MIN = 96   # stop folding at lengths <= this (or odd)
PREFETCH = 2

_prog_cache = {}


def _plan(batch_np):
    cnt = np.bincount(batch_np, minlength=B).astype(np.int64)
    starts = np.zeros(B + 1, np.int64)
    np.cumsum(cnt, out=starts[1:])
    nz = [g for g in range(B) if cnt[g] > 0]
    order = sorted(nz, key=lambda g: (-int(cnt[g]), g))
    percore = [[] for _ in range(NCORES)]
    for i, g in enumerate(order):
        r, k = divmod(i, NCORES)
        if r % 2:
            k = NCORES - 1 - k
        percore[k].append(g)
    nslot = max(len(p) for p in percore)
    slot_len = []
    for j in range(nslot):
        m = 0
        for p in percore:
            if j < len(p):
                m = max(m, -(-int(cnt[p[j]]) // 128) * 128)
        assert m <= CHUNK_MAX, f"graph too large for chunk: {m}"
        slot_len.append(m)
    # slot_len is non-increasing by construction
    slot_off = []
    off = 0
    for L in slot_len:
        slot_off.append(off)
        off += L
    T = off
    chunks = []  # (first_slot, nslots, chunk_off, chunk_len)
    cur0, cur_len = 0, 0
    for j in range(nslot):
        # keep the first chunk small: it gates pipeline fill
        cap = FIRST_CAP if not chunks else CHUNK_MAX
        if cur_len and cur_len + slot_len[j] > cap:
            chunks.append((cur0, j - cur0, slot_off[cur0], cur_len))
            cur0, cur_len = j, 0
        cur_len += slot_len[j]
    if cur_len:
        chunks.append((cur0, nslot - cur0, slot_off[cur0], cur_len))
    # split a small tail off the last chunk so the pipeline drains fast
    (s0, ns, coff, clen) = chunks[-1]
    if ns > 2 and clen > 2 * LAST_CAP:
        cut, cut_len = ns, 0
        while cut > 1 and cut_len + slot_len[s0 + cut - 1] <= LAST_CAP:
            cut -= 1
            cut_len += slot_len[s0 + cut]
        if 0 < cut < ns:
            chunks[-1] = (s0, cut, coff, clen - cut_len)
            chunks.append((s0 + cut, ns - cut, slot_off[s0 + cut], cut_len))
    return cnt, starts, percore, slot_len, slot_off, chunks, T


def _build(slot_len, slot_off, chunks, T):
    nslot = len(slot_len)
    A = mybir.AluOpType
    nc = bacc.Bacc()
    xcm = nc.dram_tensor("xcm", [128, T], f16, kind="ExternalInput")
    invr = nc.dram_tensor("invr", [128, 2 * nslot], f32, kind="ExternalInput")
    pb = nc.dram_tensor("pb", [128, 4], f32, kind="ExternalInput")
    outp = nc.dram_tensor("outp", [128, T], f16, kind="ExternalOutput")

    nchunk = len(chunks)

    with tile.TileContext(nc) as tc:
        with tc.tile_pool(name="const", bufs=1) as constp, \
             tc.tile_pool(name="dpool", bufs=PREFETCH + 1) as dpool, \
             tc.tile_pool(name="opool", bufs=2) as opool, \
             tc.tile_pool(name="scrp", bufs=1) as scrp, \
             tc.tile_pool(name="statp", bufs=3) as statp:

            invt = constp.tile([128, 2 * nslot], f32)
            nc.sync.dma_start(out=invt, in_=invr.ap()[:, :])
            pbt = constp.tile([128, 4], f32)
            nc.sync.dma_start(out=pbt, in_=pb.ap()[:, :])
            epst = constp.tile([128, 1], f32)
            nc.vector.memset(epst, EPS)
            w_col = pbt[:, 0:1]
            b_col = pbt[:, 1:2]
            negs_col = pbt[:, 2:3]
            coef_col = pbt[:, 3:4]

            SCR = scrp.tile([128, CHUNK_MAX], f16)

            Dt = [None] * nchunk

            def load(c):
                (s0, ns, coff, clen) = chunks[c]
                D = dpool.tile([128, CHUNK_MAX], f16, tag="D")
                nc.sync.dma_start(out=D[:, 0:clen],
                                  in_=xcm.ap()[:, coff:coff + clen])
                Dt[c] = D

            for c in range(min(PREFETCH + 1, nchunk)):
                load(c)

            for c, (s0, ns, coff, clen) in enumerate(chunks):
                D = Dt[c]
                OUT = opool.tile([128, CHUNK_MAX], f16, tag="OUT")
                sums = statp.tile([128, 2 * ns], f32, tag="sums")
                sumx = sums[:, 0:ns]
                sumx2 = sums[:, ns:2 * ns]

                # ---- sum(x): per equal-length slot group, fold tree on DVE
                i = 0
                while i < ns:
                    L = slot_len[s0 + i]
                    j = i
                    while j < ns and slot_len[s0 + j] == L:
                        j += 1
                    gs = j - i  # group size
                    a = slot_off[s0 + i] - coff
                    src = D[:, a:a + gs * L].rearrange("p (s l) -> p s l", l=L)
                    scr_off = 0
                    Lc = L
                    while Lc > FOLD_MIN and Lc % 2 == 0:
                        h = Lc // 2
                        dst = SCR[:, scr_off:scr_off + gs * h].rearrange(
                            "p (s l) -> p s l", l=h)
                        nc.vector.tensor_tensor(
                            out=dst, in0=src[:, :, 0:h], in1=src[:, :, h:Lc],
                            op=A.add)
                        src = dst
                        scr_off += gs * h
                        Lc = h
                    nc.vector.tensor_reduce(
                        out=sumx[:, i:j], in_=src,
                        axis=mybir.AxisListType.X, op=A.add)
                    i = j

                # ---- sum(x^2): ACT Square + accumulate, per slot
                for i in range(ns):
                    a = slot_off[s0 + i] - coff
                    e = a + slot_len[s0 + i]
                    nc.scalar.activation(
                        out=OUT[:, a:e], in_=D[:, a:e],
                        func=mybir.ActivationFunctionType.Square,
                        accum_out=sumx2[:, i:i + 1])

                # ---- stats -> per-slot affine maps W, B ([128, ns] f32)
                st = statp.tile([128, 4 * ns], f32, tag="st")
                mom = st[:, 0:2 * ns]       # [mean | ex2]
                mean = st[:, 0:ns]
                ex2 = st[:, ns:2 * ns]
                t2 = st[:, 2 * ns:3 * ns]
                istd = st[:, 3 * ns:4 * ns]
                wb = statp.tile([128, 2 * ns], f32, tag="wb")
                Wt = wb[:, 0:ns]
                Bt = wb[:, ns:2 * ns]

                # [mean|ex2] = [sumx|sumx2] * [inv|inv] (invr holds inv twice)
                inv2 = invt.rearrange("p (h n) -> p h n",
                                      n=nslot)[:, :, s0:s0 + ns]
                nc.vector.tensor_tensor(
                    out=mom.rearrange("p (h n) -> p h n", n=ns),
                    in0=sums.rearrange("p (h n) -> p h n", n=ns),
                    in1=inv2, op=A.mult)
                nc.vector.tensor_tensor(out=t2, in0=mean, in1=mean, op=A.mult)
                nc.vector.tensor_scalar(out=t2, in0=t2, scalar1=coef_col,
                                        scalar2=None, op0=A.mult)
                nc.vector.tensor_tensor(out=t2, in0=ex2, in1=t2,
                                        op=A.subtract)
                nc.scalar.activation(out=t2, in_=t2,
                                     func=mybir.ActivationFunctionType.Sqrt,
                                     bias=epst)
                nc.vector.reciprocal(out=istd, in_=t2)
                nc.vector.tensor_scalar(out=Wt, in0=istd, scalar1=w_col,
                                        scalar2=None, op0=A.mult)
                nc.vector.tensor_tensor(out=t2, in0=mean, in1=Wt, op=A.mult)
                nc.vector.tensor_scalar(out=Bt, in0=t2, scalar1=negs_col,
                                        scalar2=b_col, op0=A.mult, op1=A.add)

                # ---- fused apply: out = x*W[slot] + B[slot]
                for i in range(ns):
                    a = slot_off[s0 + i] - coff
                    e = a + slot_len[s0 + i]
                    nc.vector.tensor_scalar(
                        out=OUT[:, a:e], in0=D[:, a:e],
                        scalar1=Wt[:, i:i + 1], scalar2=Bt[:, i:i + 1],
                        op0=A.mult, op1=A.add)
                nc.sync.dma_start(out=outp.ap()[:, coff:coff + clen],
                                  in_=OUT[:, 0:clen])
                if c + PREFETCH + 1 < nchunk:
                    load(c + PREFETCH + 1)

    nc.finalize()
    return nc


def kernel(x, batch, weight, bias, mean_scale, batch_size):
    x = np.asarray(x, dtype=np.float32)
    batch_np = np.asarray(batch).astype(np.int64)
    w = np.asarray(weight, dtype=np.float32)
    b = np.asarray(bias, dtype=np.float32)
    s = np.asarray(mean_scale, dtype=np.float32)
    assert x.shape == (N, C) and int(batch_size) == B

    cnt, starts, percore, slot_len, slot_off, chunks, T = _plan(batch_np)
    nslot = len(slot_len)

    key = tuple(slot_len)
    if key not in _prog_cache:
        _prog_cache[key] = _build(slot_len, slot_off, chunks, T)
    nc = _prog_cache[key]

    x16 = x.astype(np.float16)
    pbm = np.ascontiguousarray(
        np.stack([w, b, -s, 2.0 * s - s * s], axis=1), dtype=np.float32)

    in_maps = []
    for k in range(NCORES):
        xb = np.zeros((T, C), np.float16)
        invm = np.zeros((2 * nslot,), np.float32)
        for j, g in enumerate(percore[k]):
            a = int(starts[g])
            n = int(cnt[g])
            o = slot_off[j]
            xb[o:o + n] = x16[a:a + n]
            invm[j] = 1.0 / n
            invm[nslot + j] = 1.0 / n
        xcm_np = np.ascontiguousarray(xb.T)
        inv128 = np.ascontiguousarray(
            np.broadcast_to(invm[None, :], (128, 2 * nslot)), dtype=np.float32)
        in_maps.append({"xcm": xcm_np, "invr": inv128, "pb": pbm})

    import os
    kw = {}
    if os.environ.get("GN_TRACE", "0") == "1":
        kw = {"trace": True,
              "tmpdir": os.environ.get("GN_TRACE_DIR") or None}
    res = run_bass_kernel_spmd(nc, in_maps, core_ids=list(range(NCORES)), **kw)
    global last_results
    last_results = res

    out = np.empty((N, C), np.float32)
    for k in range(NCORES):
        op = np.asarray(res.results[k]["outp"])  # [128, T] f16
        opT = np.ascontiguousarray(op.T)
        for j, g in enumerate(percore[k]):
            a = int(starts[g])
            n = int(cnt[g])
            o = slot_off[j]
            out[a:a + n] = opT[o:o + n]
    return out
